# revision 19
# baseline (speedup 1.0000x reference)
"""BoxConv2d Trainium2 kernel (8 NeuronCores, SPMD).

Math: the reference's integral-image + fractional box-edge interpolation
pipeline is linear in the input and separable, so per output channel
k = (c, f) it collapses to two dense 128x128 matrix products:

    out[b,k] = A_k @ x[b,c] @ B_k^T

with banded "pixel overlap" matrices
    A_k[xo, a] = clamp(xo - a + x_max_k + 1, 0, 1)
                 - clamp(xo - a + x_min_k, 0, 1)
and likewise B_k for columns.  A/B are built on the host from the tiny
(C,F) box params; the device does pure 128-contraction matmuls.

Sharding: the K = C*F = 128 output channels are split across 8 cores
(16 channels = 4 in_planes per core), so each core reads only its own
4 input planes and input reads are not duplicated chip-wide.

Device dataflow per core (all operands bfloat16, PSUM accum fp32):
  pass 1 (per b,c):   V[j, (f,xo)]  = x_bc^T A^T  (lhsT=x_bc, N=512)
  pass 2 (per c,f,h): O[yo, (bh,xo)] = B_k V      (lhsT=B_k^T, N=512)

Schedule (v2, rebuilt from trace analysis of v1):
  - The PE streams warm back-to-back MMs at ~216ns (N/2.4+2.5); all v1
    losses were elsewhere: input DMA landing at 15.5us starved the PE
    (2.3us stall -> HAM re-throttle -> 630ns cold MMs), output DMA was
    serialized on one queue starting at 21us, and the PSUM->SBUF drains
    (the true steady-state bottleneck: 32768 fp32 columns at ~1ns/col
    split over the only two PSUM-reading engines) ran behind 50 small
    copy instructions.
  - v2: input DMAs are the first instructions, spread over the sync/
    scalar/vector hardware queues so x0+at0 land ~2.5us in; pass-1
    writes b-pairs into 2-bank PSUM tiles and pass-2 (c,f) pairs into
    2-bank tiles (2+2 bufs = all 8 banks, double buffered) so every
    drain moves 1024 columns; drains alternate Vector/Scalar; output
    leaves per (c, f-pair) as 512KB DMAs (4KB lines) on sync (+gpsimd
    for two mid chunks) as soon as both halves drain.  Pass1(c) and
    pass2(c-1) chunks interleave 1:1 to keep the PE warm.
  - Two fp32 dummy matmuls bridge the DMA wait so the HAM full-duty
    grant lands before the real stream.

Numerics: bf16 gives l2 rel error ~3e-3 vs the fp32 reference
(budget 2e-2).
"""

import sys

if "/opt/trn_rl_repo" not in sys.path:
    sys.path.insert(0, "/opt/trn_rl_repo")

import numpy as np
import ml_dtypes

import concourse.bass as bass  # noqa: F401
import concourse.mybir as mybir
import concourse.tile as tile
from concourse import bacc
from concourse.bass_utils import run_bass_kernel_spmd

B, C, F, H, W = 8, 32, 4, 128, 128
NCORES = 8
CPC = C // NCORES   # in_planes per core
KPC = CPC * F       # output channels per core
NP = B // 2         # x batch-pairs per core

_MM_DT = mybir.dt.bfloat16
_NP_DT = ml_dtypes.bfloat16

_NC_CACHE = {}
LAST_RESULT = None


def _build_nc():
    nc = bacc.Bacc(
        "TRN2", target_bir_lowering=False, debug=False, num_devices=NCORES
    )
    # x[p, a, (b2, c, j)]: batch-pairs give 4KB DMA lines
    x_p = nc.declare_dram_parameter(
        "x", [NP, H, 2 * CPC * W], _MM_DT, isOutput=False)
    # at[cp, a, (c2, f, xo)] / bt[cp, j, (c2, f, yo)]: c-pair tiles
    at_p = nc.declare_dram_parameter(
        "at", [CPC // 2, H, 2 * F * H], _MM_DT, isOutput=False)
    bt_p = nc.declare_dram_parameter(
        "bt", [CPC // 2, W, 2 * F * W], _MM_DT, isOutput=False)
    # transposed output: outT[yo, c, h, f, (bl, xo)] = out[4h+bl, c*F+f, xo, yo]
    # -> per-(c, f-pair, h) DMA writes 2KB contiguous per yo line
    out_p = nc.declare_dram_parameter(
        "outT", [W, CPC, 2, F, B * H // 2], _MM_DT, isOutput=True)

    HB = B * H // 2  # 512: half-batch column count, one PSUM bank

    with tile.TileContext(nc) as tc:
        with (
            tc.tile_pool(name="const", bufs=1) as cpool,
            tc.tile_pool(name="xin", bufs=NP) as xpool,
            tc.tile_pool(name="vall", bufs=4) as vpool,
            tc.tile_pool(name="osb", bufs=4) as opool,
            tc.tile_pool(name="pv", bufs=2, space="PSUM") as pvpool,
            tc.tile_pool(name="po", bufs=2, space="PSUM") as popool,
        ):
            # PSUM->SBUF drains alternate between the two engines that
            # can read PSUM
            eng_i = [0]

            def copy(dst, src):
                if eng_i[0] % 2:
                    nc.scalar.copy(dst, src)
                else:
                    nc.vector.tensor_copy(dst, src)
                eng_i[0] += 1

            # input DMAs are the very first instructions: three hardware
            # queues in parallel, each queue's first transfer is one the
            # pipeline needs first (x0 / x1 / at0).  scalar+vector finish
            # issuing (~1.4us) long before their first drains (~3.5us).
            at_sb = [None] * (CPC // 2)
            bt_sb = [None] * (CPC // 2)
            x_sb = [None] * NP

            def load_x(p, eng):
                x_sb[p] = xpool.tile(
                    [128, 2 * CPC * W], _MM_DT, name=f"xsb{p}", tag="x"
                )
                eng.dma_start(x_sb[p][:], x_p[p])

            def load_at(cp, eng):
                at_sb[cp] = cpool.tile([128, 2 * F * H], _MM_DT,
                                       name=f"at{cp}", tag=f"at{cp}")
                eng.dma_start(at_sb[cp][:], at_p[cp])

            def load_bt(cp, eng):
                bt_sb[cp] = cpool.tile([128, 2 * F * W], _MM_DT,
                                       name=f"bt{cp}", tag=f"bt{cp}")
                eng.dma_start(bt_sb[cp][:], bt_p[cp])

            # per-ring DMA throughput is only ~110-150GB/s and each ring
            # drains serially, so the three first-needed tiles (x0, x1,
            # at0) each head a different ring, and x2/x3 ride second
            # slots; bt/at1 (needed later) fill the remaining slots
            load_x(1, nc.sync)
            load_x(0, nc.scalar)
            load_at(0, nc.gpsimd)
            load_x(2, nc.sync)
            load_x(3, nc.scalar)
            load_bt(0, nc.gpsimd)
            load_bt(1, nc.sync)
            load_at(1, nc.scalar)

            # warm-up: dummy fp32 matmuls (higher switching activity than
            # bf16) start the HAM activity clock during the input DMA
            # window, so the full-duty grant lands as early as possible
            # into the real matmul stream.  memset on Vector (idle until
            # the first drain); dummies target the po pool so the pass-1
            # PSUM path stays clean.
            dum = cpool.tile([128, HB], mybir.dt.float32,
                             name="dum", tag="dum")
            nc.vector.memset(dum[:], 0.0)

            def emit_dummy(i):
                d_ps = popool.tile([128, 2 * HB], mybir.dt.float32,
                                   name=f"dps{i}", tag="po")
                nc.tensor.matmul(
                    d_ps[:, :HB],
                    lhsT=dum[:, :W],
                    rhs=dum[:],
                    start=True,
                    stop=True,
                )

            emit_dummy(0)
            emit_dummy(1)

            v_full = [None] * CPC

            def emit_v_chunk(c, p):
                # pass1: both b's of pair p for plane c into one 2-bank
                # PSUM tile, then a single 1024-col drain scattered into
                # V[j, (f, b, xo)]
                if p == 0:
                    v_full[c] = vpool.tile([128, F * B * H], _MM_DT,
                                           name=f"vall{c}", tag="vall")
                v_ps = pvpool.tile([128, 2 * F * H], mybir.dt.float32,
                                   name=f"vps{c}{p}", tag="pv")
                for hb in range(2):
                    nc.tensor.matmul(
                        v_ps[:, hb * F * H:(hb + 1) * F * H],
                        lhsT=x_sb[p][:, (hb * CPC + c) * W:
                                     (hb * CPC + c + 1) * W],
                        rhs=at_sb[c // 2][:, (c % 2) * F * H:
                                          (c % 2 + 1) * F * H],
                        start=True,
                        stop=True,
                    )
                # dst iterated (b2, f, xo) to match the PSUM layout
                dst = v_full[c][:].rearrange(
                    "p (f b xo) -> p b f xo", f=F, b=B
                )[:, 2 * p:2 * p + 2]
                copy(dst, v_ps[:])

            oq_i = [0]

            def out_eng():
                # odd slots on sync so the tail-critical last chunk rides
                # the fast queue; gpsimd (slower ring) takes even slots
                eng = nc.gpsimd if oq_i[0] % 2 == 0 else nc.sync
                oq_i[0] += 1
                return eng

            def emit_o_chunk(c, fp, h, split_tail=False):
                # pass2: O[yo, (bl,xo)] for batch-half h of the f-pair
                # (fp, fp+1) of plane c; the half-split matters because
                # h=0 depends only on V pairs p0/p1 (i.e. x0/x1), so
                # pass-2 work starts while x2/x3 are still in flight.
                # Two matmuls fill a 2-bank PSUM tile, one 1024-col
                # drain, one 256KB DMA (sync/gpsimd alternate).
                o_ps = popool.tile([128, B * H], mybir.dt.float32,
                                   name=f"ops{c}{fp}{h}", tag="po")
                for f2 in range(2):
                    f = fp + f2
                    nc.tensor.matmul(
                        o_ps[:, f2 * HB:(f2 + 1) * HB],
                        lhsT=bt_sb[c // 2][:, ((c % 2) * F + f) * W:
                                           ((c % 2) * F + f + 1) * W],
                        rhs=v_full[c][:, f * B * H + h * HB:
                                      f * B * H + (h + 1) * HB],
                        start=True,
                        stop=True,
                    )
                seg = opool.tile([128, B * H], _MM_DT,
                                 name=f"osb{c}{fp}{h}", tag="osb")
                if split_tail:
                    # final chunks: drain halves on both engines in
                    # parallel, shortening the post-last-matmul tail
                    nc.vector.tensor_copy(seg[:, :HB], o_ps[:, :HB])
                    nc.scalar.copy(seg[:, HB:], o_ps[:, HB:])
                else:
                    copy(seg[:], o_ps[:])
                out_eng().dma_start(out_p[:, c, h, fp:fp + 2], seg[:])

            # arrival-aware emission: the Tensor stream executes in
            # program order, so chunks are ordered by when their inputs
            # land (x pairs every ~2us, at1 last) and by V half-batch
            # readiness.  O(c,fp,h) needs only V(c) pairs {2h, 2h+1},
            # so pass-2 h=0 chunks start while x2/x3 are in flight, and
            # independent pass-1 work sits between consecutive O chunks
            # to hide the PSUM-recycle latency.
            sched = [
                ("v", 0, 0, 0), ("v", 1, 0, 0), ("d",),
                ("v", 0, 1, 0), ("v", 1, 1, 0), ("d",),
                ("v", 0, 2, 0), ("v", 1, 2, 0),
                ("o", 0, 0, 0), ("o", 0, 2, 0),
                ("v", 0, 3, 0), ("v", 1, 3, 0),
                ("o", 1, 0, 0), ("o", 1, 2, 0),
                ("o", 0, 0, 1), ("v", 2, 0, 0),
                ("o", 0, 2, 1), ("v", 2, 1, 0),
                ("o", 1, 0, 1), ("v", 3, 0, 0),
                ("o", 1, 2, 1), ("v", 2, 2, 0),
                ("o", 2, 0, 0), ("v", 3, 1, 0),
                ("o", 2, 2, 0), ("v", 2, 3, 0),
                ("o", 3, 0, 0), ("v", 3, 2, 0),
                ("o", 2, 0, 1), ("v", 3, 3, 0),
            ]
            di = [2]
            for item in sched:
                if item[0] == "v":
                    emit_v_chunk(item[1], item[2])
                elif item[0] == "o":
                    emit_o_chunk(item[1], item[2], item[3])
                else:
                    emit_dummy(di[0])
                    di[0] += 1
            # tail: split drains across both engines so po recycles at
            # half the latency and the final DMAs start sooner
            emit_o_chunk(3, 2, 0, split_tail=True)
            emit_o_chunk(2, 2, 1, split_tail=True)
            emit_o_chunk(3, 0, 1, split_tail=True)
            emit_o_chunk(3, 2, 1, split_tail=True)
    nc.finalize()
    return nc


def _get_nc():
    if "nc" not in _NC_CACHE:
        _NC_CACHE["nc"] = _build_nc()
    return _NC_CACHE["nc"]


def _overlap_mats(lo, hi):
    """(K, out, in) pixel-overlap matrices for a 128-wide axis."""
    t = np.arange(128, dtype=np.float64)
    d = t[:, None] - t[None, :]  # out - in
    lo = lo.astype(np.float64)[:, None, None]
    hi = hi.astype(np.float64)[:, None, None]
    m = np.clip(d[None] + hi + 1.0, 0.0, 1.0) - np.clip(d[None] + lo, 0.0, 1.0)
    return m.astype(np.float32)


def _make_in_maps(input, x_min, x_max, y_min, y_max):
    A = _overlap_mats(x_min.reshape(-1), x_max.reshape(-1))   # (K, xo, a)
    Bm = _overlap_mats(y_min.reshape(-1), y_max.reshape(-1))  # (K, yo, j)
    in_maps = []
    for m in range(NCORES):
        cs = slice(CPC * m, CPC * (m + 1))
        ks = slice(KPC * m, KPC * (m + 1))
        # x[p, a, (b2, c, j)]
        xm = input[:, cs]                                   # [b, c, a, j]
        xm = xm.reshape(NP, 2, CPC, H, W)
        xm = xm.transpose(0, 3, 1, 2, 4).reshape(NP, H, 2 * CPC * W)
        # at[cp, a, (c2, f, xo)] = A[k=(cp*2+c2)*F+f, xo, a]
        at = A[ks].reshape(CPC // 2, 2, F, H, H).transpose(0, 4, 1, 2, 3)
        bt = Bm[ks].reshape(CPC // 2, 2, F, W, W).transpose(0, 4, 1, 2, 3)
        in_maps.append({
            "x": np.ascontiguousarray(xm).astype(_NP_DT),
            "at": np.ascontiguousarray(
                at.reshape(CPC // 2, H, 2 * F * H)).astype(_NP_DT),
            "bt": np.ascontiguousarray(
                bt.reshape(CPC // 2, W, 2 * F * W)).astype(_NP_DT),
        })
    return in_maps


def _assemble(results):
    out = np.empty((B, C * F, H, W), np.float32)
    for m in range(NCORES):
        # outT[yo, c, h, f, (bl, xo)] -> out[(h,bl), (c,f), xo, yo]
        o = results[m]["outT"].astype(np.float32).reshape(W, CPC, 2, F, 4, H)
        o = o.transpose(2, 4, 1, 3, 5, 0).reshape(B, KPC, H, W)
        out[:, KPC * m:KPC * (m + 1)] = o
    return out


def _run(inputs, trace=False):
    global LAST_RESULT
    nc = _get_nc()
    in_maps = _make_in_maps(**inputs)
    LAST_RESULT = run_bass_kernel_spmd(
        nc, in_maps, list(range(NCORES)), trace=trace
    )
    return _assemble(LAST_RESULT.results)


def kernel(input, x_min, x_max, y_min, y_max):
    return _run({
        "input": np.asarray(input, dtype=np.float32),
        "x_min": np.asarray(x_min, dtype=np.float32),
        "x_max": np.asarray(x_max, dtype=np.float32),
        "y_min": np.asarray(y_min, dtype=np.float32),
        "y_max": np.asarray(y_max, dtype=np.float32),
    })


# revision 20
# speedup vs baseline: 1.1096x; 1.1096x over previous
"""BoxConv2d Trainium2 kernel (8 NeuronCores, SPMD).

Math: the reference's integral-image + fractional box-edge interpolation
pipeline is linear in the input and separable, so per output channel
k = (c, f) it collapses to two dense 128x128 matrix products:

    out[b,k] = A_k @ x[b,c] @ B_k^T

with banded "pixel overlap" matrices
    A_k[xo, a] = clamp(xo - a + x_max_k + 1, 0, 1)
                 - clamp(xo - a + x_min_k, 0, 1)
and likewise B_k for columns.  A/B are built on the host from the tiny
(C,F) box params; the device does pure 128-contraction matmuls.

Sharding: the K = C*F = 128 output channels are split across 8 cores
(16 channels = 4 in_planes per core), so each core reads only its own
4 input planes and input reads are not duplicated chip-wide.

Device dataflow per core (all operands bfloat16, PSUM accum fp32):
  pass 1 (per b,c):   V[j, (f,xo)]  = x_bc^T A^T  (lhsT=x_bc, N=512)
  pass 2 (per c,f,h): O[yo, (bh,xo)] = B_k V      (lhsT=B_k^T, N=512)

Schedule (v2, rebuilt from trace analysis of v1):
  - The PE streams warm back-to-back MMs at ~216ns (N/2.4+2.5); all v1
    losses were elsewhere: input DMA landing at 15.5us starved the PE
    (2.3us stall -> HAM re-throttle -> 630ns cold MMs), output DMA was
    serialized on one queue starting at 21us, and the PSUM->SBUF drains
    (the true steady-state bottleneck: 32768 fp32 columns at ~1ns/col
    split over the only two PSUM-reading engines) ran behind 50 small
    copy instructions.
  - v2: input DMAs are the first instructions, spread over the sync/
    scalar/vector hardware queues so x0+at0 land ~2.5us in; pass-1
    writes b-pairs into 2-bank PSUM tiles and pass-2 (c,f) pairs into
    2-bank tiles (2+2 bufs = all 8 banks, double buffered) so every
    drain moves 1024 columns; drains alternate Vector/Scalar; output
    leaves per (c, f-pair) as 512KB DMAs (4KB lines) on sync (+gpsimd
    for two mid chunks) as soon as both halves drain.  Pass1(c) and
    pass2(c-1) chunks interleave 1:1 to keep the PE warm.
  - Two fp32 dummy matmuls bridge the DMA wait so the HAM full-duty
    grant lands before the real stream.

Numerics: bf16 gives l2 rel error ~3e-3 vs the fp32 reference
(budget 2e-2).
"""

import sys

if "/opt/trn_rl_repo" not in sys.path:
    sys.path.insert(0, "/opt/trn_rl_repo")

import numpy as np
import ml_dtypes

import concourse.bass as bass  # noqa: F401
import concourse.mybir as mybir
import concourse.tile as tile
from concourse import bacc
from concourse.bass_utils import run_bass_kernel_spmd

B, C, F, H, W = 8, 32, 4, 128, 128
NCORES = 8
CPC = C // NCORES   # in_planes per core
KPC = CPC * F       # output channels per core
NP = B // 2         # x batch-pairs per core

_MM_DT = mybir.dt.bfloat16
_NP_DT = ml_dtypes.bfloat16

_NC_CACHE = {}
LAST_RESULT = None


def _build_nc():
    nc = bacc.Bacc(
        "TRN2", target_bir_lowering=False, debug=False, num_devices=NCORES
    )
    # x[p, a, (b2, c, j)]: batch-pairs give 4KB DMA lines
    x_p = nc.declare_dram_parameter(
        "x", [NP, H, 2 * CPC * W], _MM_DT, isOutput=False)
    # at[cp, a, (c2, f, xo)] / bt[cp, j, (c2, f, yo)]: c-pair tiles
    at_p = nc.declare_dram_parameter(
        "at", [CPC // 2, H, 2 * F * H], _MM_DT, isOutput=False)
    bt_p = nc.declare_dram_parameter(
        "bt", [CPC // 2, W, 2 * F * W], _MM_DT, isOutput=False)
    # transposed output: outT[yo, c, h, f, (bl, xo)] = out[4h+bl, c*F+f, xo, yo]
    # -> per-(c, f-pair, h) DMA writes 2KB contiguous per yo line
    out_p = nc.declare_dram_parameter(
        "outT", [W, CPC, 2, F, B * H // 2], _MM_DT, isOutput=True)

    HB = B * H // 2  # 512: half-batch column count, one PSUM bank

    with tile.TileContext(nc) as tc:
        with (
            tc.tile_pool(name="const", bufs=1) as cpool,
            tc.tile_pool(name="xin", bufs=NP) as xpool,
            tc.tile_pool(name="vall", bufs=4) as vpool,
            tc.tile_pool(name="osb", bufs=4) as opool,
            tc.tile_pool(name="pv", bufs=2, space="PSUM") as pvpool,
            tc.tile_pool(name="po", bufs=2, space="PSUM") as popool,
        ):
            # PSUM->SBUF drains alternate between the two engines that
            # can read PSUM
            eng_i = [0]

            def copy(dst, src):
                if eng_i[0] % 2:
                    nc.scalar.copy(dst, src)
                else:
                    nc.vector.tensor_copy(dst, src)
                eng_i[0] += 1

            # input DMAs are the very first instructions: three hardware
            # queues in parallel, each queue's first transfer is one the
            # pipeline needs first (x0 / x1 / at0).  scalar+vector finish
            # issuing (~1.4us) long before their first drains (~3.5us).
            at_sb = [None] * (CPC // 2)
            bt_sb = [None] * (CPC // 2)
            x_sb = [None] * NP

            def load_x(p, eng):
                x_sb[p] = xpool.tile(
                    [128, 2 * CPC * W], _MM_DT, name=f"xsb{p}", tag="x"
                )
                eng.dma_start(x_sb[p][:], x_p[p])

            def load_at(cp, eng):
                at_sb[cp] = cpool.tile([128, 2 * F * H], _MM_DT,
                                       name=f"at{cp}", tag=f"at{cp}")
                eng.dma_start(at_sb[cp][:], at_p[cp])

            def load_bt(cp, eng):
                bt_sb[cp] = cpool.tile([128, 2 * F * W], _MM_DT,
                                       name=f"bt{cp}", tag=f"bt{cp}")
                eng.dma_start(bt_sb[cp][:], bt_p[cp])

            # per-ring DMA throughput is only ~110-150GB/s and each ring
            # drains serially: first-needed tiles head the two fast
            # rings (sync/scalar); bt rides the slower gpsimd ring
            # (needed only once pass-2 starts)
            load_at(0, nc.sync)
            load_x(0, nc.scalar)
            load_x(1, nc.sync)
            load_x(2, nc.scalar)
            load_bt(0, nc.gpsimd)
            load_x(3, nc.sync)
            load_at(1, nc.scalar)
            load_bt(1, nc.gpsimd)

            # warm-up: dummy fp32 matmuls (higher switching activity than
            # bf16) start the HAM activity clock during the input DMA
            # window, so the full-duty grant lands as early as possible
            # into the real matmul stream.  memset on Vector (idle until
            # the first drain); dummies target the po pool so the pass-1
            # PSUM path stays clean.
            dum = cpool.tile([128, HB], mybir.dt.float32,
                             name="dum", tag="dum")
            nc.vector.memset(dum[:], 0.0)

            def emit_dummy(i):
                d_ps = popool.tile([128, 2 * HB], mybir.dt.float32,
                                   name=f"dps{i}", tag="po")
                nc.tensor.matmul(
                    d_ps[:, :HB],
                    lhsT=dum[:, :W],
                    rhs=dum[:],
                    start=True,
                    stop=True,
                )

            emit_dummy(0)
            emit_dummy(1)

            v_full = [None] * CPC

            def emit_v_chunk(c, p):
                # pass1: both b's of pair p for plane c into one 2-bank
                # PSUM tile, then a single 1024-col drain scattered into
                # V[j, (f, b, xo)]
                if p == 0:
                    v_full[c] = vpool.tile([128, F * B * H], _MM_DT,
                                           name=f"vall{c}", tag="vall")
                v_ps = pvpool.tile([128, 2 * F * H], mybir.dt.float32,
                                   name=f"vps{c}{p}", tag="pv")
                for hb in range(2):
                    nc.tensor.matmul(
                        v_ps[:, hb * F * H:(hb + 1) * F * H],
                        lhsT=x_sb[p][:, (hb * CPC + c) * W:
                                     (hb * CPC + c + 1) * W],
                        rhs=at_sb[c // 2][:, (c % 2) * F * H:
                                          (c % 2 + 1) * F * H],
                        start=True,
                        stop=True,
                    )
                # dst iterated (b2, f, xo) to match the PSUM layout
                dst = v_full[c][:].rearrange(
                    "p (f b xo) -> p b f xo", f=F, b=B
                )[:, 2 * p:2 * p + 2]
                copy(dst, v_ps[:])

            oq_i = [0]

            def out_eng():
                # odd slots on sync so the tail-critical last chunk rides
                # the fast queue; gpsimd (slower ring) takes even slots
                eng = nc.gpsimd if oq_i[0] % 2 == 0 else nc.sync
                oq_i[0] += 1
                return eng

            def emit_o_chunk(c, fp, h, split_tail=False):
                # pass2: O[yo, (bl,xo)] for batch-half h of the f-pair
                # (fp, fp+1) of plane c; the half-split matters because
                # h=0 depends only on V pairs p0/p1 (i.e. x0/x1), so
                # pass-2 work starts while x2/x3 are still in flight.
                # Two matmuls fill a 2-bank PSUM tile, one 1024-col
                # drain, one 256KB DMA (sync/gpsimd alternate).
                o_ps = popool.tile([128, B * H], mybir.dt.float32,
                                   name=f"ops{c}{fp}{h}", tag="po")
                for f2 in range(2):
                    f = fp + f2
                    nc.tensor.matmul(
                        o_ps[:, f2 * HB:(f2 + 1) * HB],
                        lhsT=bt_sb[c // 2][:, ((c % 2) * F + f) * W:
                                           ((c % 2) * F + f + 1) * W],
                        rhs=v_full[c][:, f * B * H + h * HB:
                                      f * B * H + (h + 1) * HB],
                        start=True,
                        stop=True,
                    )
                seg = opool.tile([128, B * H], _MM_DT,
                                 name=f"osb{c}{fp}{h}", tag="osb")
                if split_tail:
                    # final chunks: drain halves on both engines in
                    # parallel, shortening the post-last-matmul tail
                    nc.vector.tensor_copy(seg[:, :HB], o_ps[:, :HB])
                    nc.scalar.copy(seg[:, HB:], o_ps[:, HB:])
                else:
                    copy(seg[:], o_ps[:])
                out_eng().dma_start(out_p[:, c, h, fp:fp + 2], seg[:])

            # arrival-aware emission: the Tensor stream executes in
            # program order, so chunks are ordered by when their inputs
            # land (x pairs every ~2us, at1 last) and by V half-batch
            # readiness.  O(c,fp,h) needs only V(c) pairs {2h, 2h+1},
            # so pass-2 h=0 chunks start while x2/x3 are in flight, and
            # independent pass-1 work sits between consecutive O chunks
            # to hide the PSUM-recycle latency.
            sched = [
                ("v", 0, 0, 0), ("v", 1, 0, 0), ("d",),
                ("v", 0, 1, 0), ("v", 1, 1, 0), ("d",),
                ("v", 0, 2, 0), ("v", 1, 2, 0),
                ("o", 0, 0, 0), ("o", 0, 2, 0),
                ("v", 0, 3, 0), ("v", 1, 3, 0),
                ("o", 1, 0, 0), ("o", 1, 2, 0),
                ("o", 0, 0, 1), ("v", 2, 0, 0),
                ("o", 0, 2, 1), ("v", 2, 1, 0),
                ("o", 1, 0, 1), ("v", 3, 0, 0),
                ("o", 1, 2, 1), ("v", 2, 2, 0),
                ("o", 2, 0, 0), ("v", 3, 1, 0),
                ("o", 2, 2, 0), ("v", 2, 3, 0),
                ("o", 3, 0, 0), ("v", 3, 2, 0),
                ("o", 2, 0, 1), ("v", 3, 3, 0),
            ]
            di = [2]
            for item in sched:
                if item[0] == "v":
                    emit_v_chunk(item[1], item[2])
                elif item[0] == "o":
                    emit_o_chunk(item[1], item[2], item[3])
                else:
                    emit_dummy(di[0])
                    di[0] += 1
            # tail: split drains across both engines so po recycles at
            # half the latency and the final DMAs start sooner
            emit_o_chunk(3, 2, 0, split_tail=True)
            emit_o_chunk(2, 2, 1, split_tail=True)
            emit_o_chunk(3, 0, 1, split_tail=True)
            emit_o_chunk(3, 2, 1, split_tail=True)
    nc.finalize()
    return nc


def _get_nc():
    if "nc" not in _NC_CACHE:
        _NC_CACHE["nc"] = _build_nc()
    return _NC_CACHE["nc"]


def _overlap_mats(lo, hi):
    """(K, out, in) pixel-overlap matrices for a 128-wide axis."""
    t = np.arange(128, dtype=np.float64)
    d = t[:, None] - t[None, :]  # out - in
    lo = lo.astype(np.float64)[:, None, None]
    hi = hi.astype(np.float64)[:, None, None]
    m = np.clip(d[None] + hi + 1.0, 0.0, 1.0) - np.clip(d[None] + lo, 0.0, 1.0)
    return m.astype(np.float32)


def _make_in_maps(input, x_min, x_max, y_min, y_max):
    A = _overlap_mats(x_min.reshape(-1), x_max.reshape(-1))   # (K, xo, a)
    Bm = _overlap_mats(y_min.reshape(-1), y_max.reshape(-1))  # (K, yo, j)
    in_maps = []
    for m in range(NCORES):
        cs = slice(CPC * m, CPC * (m + 1))
        ks = slice(KPC * m, KPC * (m + 1))
        # x[p, a, (b2, c, j)]
        xm = input[:, cs]                                   # [b, c, a, j]
        xm = xm.reshape(NP, 2, CPC, H, W)
        xm = xm.transpose(0, 3, 1, 2, 4).reshape(NP, H, 2 * CPC * W)
        # at[cp, a, (c2, f, xo)] = A[k=(cp*2+c2)*F+f, xo, a]
        at = A[ks].reshape(CPC // 2, 2, F, H, H).transpose(0, 4, 1, 2, 3)
        bt = Bm[ks].reshape(CPC // 2, 2, F, W, W).transpose(0, 4, 1, 2, 3)
        in_maps.append({
            "x": np.ascontiguousarray(xm).astype(_NP_DT),
            "at": np.ascontiguousarray(
                at.reshape(CPC // 2, H, 2 * F * H)).astype(_NP_DT),
            "bt": np.ascontiguousarray(
                bt.reshape(CPC // 2, W, 2 * F * W)).astype(_NP_DT),
        })
    return in_maps


def _assemble(results):
    out = np.empty((B, C * F, H, W), np.float32)
    for m in range(NCORES):
        # outT[yo, c, h, f, (bl, xo)] -> out[(h,bl), (c,f), xo, yo]
        o = results[m]["outT"].astype(np.float32).reshape(W, CPC, 2, F, 4, H)
        o = o.transpose(2, 4, 1, 3, 5, 0).reshape(B, KPC, H, W)
        out[:, KPC * m:KPC * (m + 1)] = o
    return out


def _run(inputs, trace=False):
    global LAST_RESULT
    nc = _get_nc()
    in_maps = _make_in_maps(**inputs)
    LAST_RESULT = run_bass_kernel_spmd(
        nc, in_maps, list(range(NCORES)), trace=trace
    )
    return _assemble(LAST_RESULT.results)


def kernel(input, x_min, x_max, y_min, y_max):
    return _run({
        "input": np.asarray(input, dtype=np.float32),
        "x_min": np.asarray(x_min, dtype=np.float32),
        "x_max": np.asarray(x_max, dtype=np.float32),
        "y_min": np.asarray(y_min, dtype=np.float32),
        "y_max": np.asarray(y_max, dtype=np.float32),
    })


# revision 21
# speedup vs baseline: 1.1178x; 1.0073x over previous
"""BoxConv2d Trainium2 kernel (8 NeuronCores, SPMD).

Math: the reference's integral-image + fractional box-edge interpolation
pipeline is linear in the input and separable, so per output channel
k = (c, f) it collapses to two dense 128x128 matrix products:

    out[b,k] = A_k @ x[b,c] @ B_k^T

with banded "pixel overlap" matrices
    A_k[xo, a] = clamp(xo - a + x_max_k + 1, 0, 1)
                 - clamp(xo - a + x_min_k, 0, 1)
and likewise B_k for columns.  A/B are built on the host from the tiny
(C,F) box params; the device does pure 128-contraction matmuls.

Sharding: the K = C*F = 128 output channels are split across 8 cores
(16 channels = 4 in_planes per core), so each core reads only its own
4 input planes and input reads are not duplicated chip-wide.

Device dataflow per core (all operands bfloat16, PSUM accum fp32):
  pass 1 (per b,c):   V[j, (f,xo)]  = x_bc^T A^T  (lhsT=x_bc, N=512)
  pass 2 (per c,f,h): O[yo, (bh,xo)] = B_k V      (lhsT=B_k^T, N=512)

Schedule (v2, rebuilt from trace analysis of v1):
  - The PE streams warm back-to-back MMs at ~216ns (N/2.4+2.5); all v1
    losses were elsewhere: input DMA landing at 15.5us starved the PE
    (2.3us stall -> HAM re-throttle -> 630ns cold MMs), output DMA was
    serialized on one queue starting at 21us, and the PSUM->SBUF drains
    (the true steady-state bottleneck: 32768 fp32 columns at ~1ns/col
    split over the only two PSUM-reading engines) ran behind 50 small
    copy instructions.
  - v2: input DMAs are the first instructions, spread over the sync/
    scalar/vector hardware queues so x0+at0 land ~2.5us in; pass-1
    writes b-pairs into 2-bank PSUM tiles and pass-2 (c,f) pairs into
    2-bank tiles (2+2 bufs = all 8 banks, double buffered) so every
    drain moves 1024 columns; drains alternate Vector/Scalar; output
    leaves per (c, f-pair) as 512KB DMAs (4KB lines) on sync (+gpsimd
    for two mid chunks) as soon as both halves drain.  Pass1(c) and
    pass2(c-1) chunks interleave 1:1 to keep the PE warm.
  - Two fp32 dummy matmuls bridge the DMA wait so the HAM full-duty
    grant lands before the real stream.

Numerics: bf16 gives l2 rel error ~3e-3 vs the fp32 reference
(budget 2e-2).
"""

import sys

if "/opt/trn_rl_repo" not in sys.path:
    sys.path.insert(0, "/opt/trn_rl_repo")

import numpy as np
import ml_dtypes

import concourse.bass as bass  # noqa: F401
import concourse.mybir as mybir
import concourse.tile as tile
from concourse import bacc
from concourse.bass_utils import run_bass_kernel_spmd

B, C, F, H, W = 8, 32, 4, 128, 128
NCORES = 8
CPC = C // NCORES   # in_planes per core
KPC = CPC * F       # output channels per core
NP = B // 2         # x batch-pairs per core

_MM_DT = mybir.dt.bfloat16
_NP_DT = ml_dtypes.bfloat16

_NC_CACHE = {}
LAST_RESULT = None


def _build_nc():
    nc = bacc.Bacc(
        "TRN2", target_bir_lowering=False, debug=False, num_devices=NCORES
    )
    # x[p, a, (b2, c, j)]: batch-pairs give 4KB DMA lines
    x_p = nc.declare_dram_parameter(
        "x", [NP, H, 2 * CPC * W], _MM_DT, isOutput=False)
    # at[cp, a, (c2, f, xo)] / bt[cp, j, (c2, f, yo)]: c-pair tiles
    at_p = nc.declare_dram_parameter(
        "at", [CPC // 2, H, 2 * F * H], _MM_DT, isOutput=False)
    bt_p = nc.declare_dram_parameter(
        "bt", [CPC // 2, W, 2 * F * W], _MM_DT, isOutput=False)
    # transposed output: outT[yo, c, h, f, (bl, xo)] = out[4h+bl, c*F+f, xo, yo]
    # -> per-(c, f-pair, h) DMA writes 2KB contiguous per yo line
    out_p = nc.declare_dram_parameter(
        "outT", [W, CPC, 2, F, B * H // 2], _MM_DT, isOutput=True)

    HB = B * H // 2  # 512: half-batch column count, one PSUM bank

    with tile.TileContext(nc) as tc:
        with (
            tc.tile_pool(name="const", bufs=1) as cpool,
            tc.tile_pool(name="xin", bufs=NP) as xpool,
            tc.tile_pool(name="vall", bufs=4) as vpool,
            tc.tile_pool(name="osb", bufs=4) as opool,
            tc.tile_pool(name="pv", bufs=2, space="PSUM") as pvpool,
            tc.tile_pool(name="po", bufs=2, space="PSUM") as popool,
        ):
            # PSUM->SBUF drains alternate between the two engines that
            # can read PSUM
            eng_i = [0]

            def copy(dst, src):
                if eng_i[0] % 2:
                    nc.scalar.copy(dst, src)
                else:
                    nc.vector.tensor_copy(dst, src)
                eng_i[0] += 1

            # input DMAs are the very first instructions: three hardware
            # queues in parallel, each queue's first transfer is one the
            # pipeline needs first (x0 / x1 / at0).  scalar+vector finish
            # issuing (~1.4us) long before their first drains (~3.5us).
            at_sb = [None] * (CPC // 2)
            bt_sb = [None] * (CPC // 2)
            x_sb = [None] * NP

            def load_x(p, eng):
                x_sb[p] = xpool.tile(
                    [128, 2 * CPC * W], _MM_DT, name=f"xsb{p}", tag="x"
                )
                eng.dma_start(x_sb[p][:], x_p[p])

            def load_at(cp, eng):
                at_sb[cp] = cpool.tile([128, 2 * F * H], _MM_DT,
                                       name=f"at{cp}", tag=f"at{cp}")
                eng.dma_start(at_sb[cp][:], at_p[cp])

            def load_bt(cp, eng):
                bt_sb[cp] = cpool.tile([128, 2 * F * W], _MM_DT,
                                       name=f"bt{cp}", tag=f"bt{cp}")
                eng.dma_start(bt_sb[cp][:], bt_p[cp])

            # per-ring DMA throughput is only ~110-150GB/s and each ring
            # drains serially: first-needed tiles head the two fast
            # rings (sync/scalar); bt rides the slower gpsimd ring
            # (needed only once pass-2 starts)
            load_at(0, nc.sync)
            load_x(0, nc.scalar)
            load_x(1, nc.sync)
            load_x(2, nc.scalar)
            load_bt(0, nc.gpsimd)
            load_x(3, nc.sync)
            load_at(1, nc.scalar)
            load_bt(1, nc.gpsimd)

            # warm-up: dummy fp32 matmuls (higher switching activity than
            # bf16) start the HAM activity clock during the input DMA
            # window, so the full-duty grant lands as early as possible
            # into the real matmul stream.  memset on Vector (idle until
            # the first drain); dummies target the po pool so the pass-1
            # PSUM path stays clean.
            dum = cpool.tile([128, HB], mybir.dt.float32,
                             name="dum", tag="dum")
            nc.vector.memset(dum[:], 0.0)

            def emit_dummy(i):
                d_ps = popool.tile([128, 2 * HB], mybir.dt.float32,
                                   name=f"dps{i}", tag="po")
                nc.tensor.matmul(
                    d_ps[:, :HB],
                    lhsT=dum[:, :W],
                    rhs=dum[:],
                    start=True,
                    stop=True,
                )

            emit_dummy(0)
            emit_dummy(1)

            v_full = [None] * CPC

            def emit_v_chunk(c, p):
                # pass1: both b's of pair p for plane c into one 2-bank
                # PSUM tile, then a single 1024-col drain scattered into
                # V[j, (f, b, xo)]
                if p == 0:
                    v_full[c] = vpool.tile([128, F * B * H], _MM_DT,
                                           name=f"vall{c}", tag="vall")
                v_ps = pvpool.tile([128, 2 * F * H], mybir.dt.float32,
                                   name=f"vps{c}{p}", tag="pv")
                for hb in range(2):
                    nc.tensor.matmul(
                        v_ps[:, hb * F * H:(hb + 1) * F * H],
                        lhsT=x_sb[p][:, (hb * CPC + c) * W:
                                     (hb * CPC + c + 1) * W],
                        rhs=at_sb[c // 2][:, (c % 2) * F * H:
                                          (c % 2 + 1) * F * H],
                        start=True,
                        stop=True,
                    )
                # dst iterated (b2, f, xo) to match the PSUM layout
                dst = v_full[c][:].rearrange(
                    "p (f b xo) -> p b f xo", f=F, b=B
                )[:, 2 * p:2 * p + 2]
                copy(dst, v_ps[:])

            oq_i = [0]

            def out_eng():
                # odd slots on sync so the tail-critical last chunk rides
                # the fast queue; gpsimd (slower ring) takes even slots
                eng = nc.gpsimd if oq_i[0] % 2 == 0 else nc.sync
                oq_i[0] += 1
                return eng

            def emit_o_chunk(c, fp, h, split_tail=False):
                # pass2: O[yo, (bl,xo)] for batch-half h of the f-pair
                # (fp, fp+1) of plane c; the half-split matters because
                # h=0 depends only on V pairs p0/p1 (i.e. x0/x1), so
                # pass-2 work starts while x2/x3 are still in flight.
                # Two matmuls fill a 2-bank PSUM tile, one 1024-col
                # drain, one 256KB DMA (sync/gpsimd alternate).
                o_ps = popool.tile([128, B * H], mybir.dt.float32,
                                   name=f"ops{c}{fp}{h}", tag="po")
                for f2 in range(2):
                    f = fp + f2
                    nc.tensor.matmul(
                        o_ps[:, f2 * HB:(f2 + 1) * HB],
                        lhsT=bt_sb[c // 2][:, ((c % 2) * F + f) * W:
                                           ((c % 2) * F + f + 1) * W],
                        rhs=v_full[c][:, f * B * H + h * HB:
                                      f * B * H + (h + 1) * HB],
                        start=True,
                        stop=True,
                    )
                seg = opool.tile([128, B * H], _MM_DT,
                                 name=f"osb{c}{fp}{h}", tag="osb")
                if split_tail:
                    # final chunks: drain halves on both engines in
                    # parallel, shortening the post-last-matmul tail
                    nc.vector.tensor_copy(seg[:, :HB], o_ps[:, :HB])
                    nc.scalar.copy(seg[:, HB:], o_ps[:, HB:])
                else:
                    copy(seg[:], o_ps[:])
                out_eng().dma_start(out_p[:, c, h, fp:fp + 2], seg[:])

            # arrival-aware emission: the Tensor stream executes in
            # program order, so chunks are ordered by when their inputs
            # land (x pairs every ~2us, at1 last) and by V half-batch
            # readiness.  O(c,fp,h) needs only V(c) pairs {2h, 2h+1},
            # so pass-2 h=0 chunks start while x2/x3 are in flight, and
            # independent pass-1 work sits between consecutive O chunks
            # to hide the PSUM-recycle latency.
            sched = [
                ("v", 0, 0, 0), ("v", 1, 0, 0), ("d",),
                ("v", 0, 1, 0), ("v", 1, 1, 0), ("d",),
                ("o", 0, 0, 0),
                ("v", 0, 2, 0), ("o", 0, 2, 0),
                ("v", 1, 2, 0), ("o", 1, 0, 0),
                ("v", 0, 3, 0), ("o", 1, 2, 0),
                ("v", 1, 3, 0), ("o", 0, 0, 1),
                ("v", 2, 0, 0), ("o", 0, 2, 1),
                ("v", 2, 1, 0), ("o", 1, 0, 1),
                ("v", 3, 0, 0), ("o", 1, 2, 1),
                ("v", 2, 2, 0), ("o", 2, 0, 0),
                ("v", 3, 1, 0), ("o", 2, 2, 0),
                ("v", 2, 3, 0), ("o", 3, 0, 0),
                ("v", 3, 2, 0), ("o", 2, 0, 1),
                ("v", 3, 3, 0),
            ]
            di = [2]
            for item in sched:
                if item[0] == "v":
                    emit_v_chunk(item[1], item[2])
                elif item[0] == "o":
                    emit_o_chunk(item[1], item[2], item[3])
                else:
                    emit_dummy(di[0])
                    di[0] += 1
            # tail: split drains across both engines so po recycles at
            # half the latency and the final DMAs start sooner
            emit_o_chunk(3, 2, 0, split_tail=True)
            emit_o_chunk(2, 2, 1, split_tail=True)
            emit_o_chunk(3, 0, 1, split_tail=True)
            emit_o_chunk(3, 2, 1, split_tail=True)
    nc.finalize()
    return nc


def _get_nc():
    if "nc" not in _NC_CACHE:
        _NC_CACHE["nc"] = _build_nc()
    return _NC_CACHE["nc"]


def _overlap_mats(lo, hi):
    """(K, out, in) pixel-overlap matrices for a 128-wide axis."""
    t = np.arange(128, dtype=np.float64)
    d = t[:, None] - t[None, :]  # out - in
    lo = lo.astype(np.float64)[:, None, None]
    hi = hi.astype(np.float64)[:, None, None]
    m = np.clip(d[None] + hi + 1.0, 0.0, 1.0) - np.clip(d[None] + lo, 0.0, 1.0)
    return m.astype(np.float32)


def _make_in_maps(input, x_min, x_max, y_min, y_max):
    A = _overlap_mats(x_min.reshape(-1), x_max.reshape(-1))   # (K, xo, a)
    Bm = _overlap_mats(y_min.reshape(-1), y_max.reshape(-1))  # (K, yo, j)
    in_maps = []
    for m in range(NCORES):
        cs = slice(CPC * m, CPC * (m + 1))
        ks = slice(KPC * m, KPC * (m + 1))
        # x[p, a, (b2, c, j)]
        xm = input[:, cs]                                   # [b, c, a, j]
        xm = xm.reshape(NP, 2, CPC, H, W)
        xm = xm.transpose(0, 3, 1, 2, 4).reshape(NP, H, 2 * CPC * W)
        # at[cp, a, (c2, f, xo)] = A[k=(cp*2+c2)*F+f, xo, a]
        at = A[ks].reshape(CPC // 2, 2, F, H, H).transpose(0, 4, 1, 2, 3)
        bt = Bm[ks].reshape(CPC // 2, 2, F, W, W).transpose(0, 4, 1, 2, 3)
        in_maps.append({
            "x": np.ascontiguousarray(xm).astype(_NP_DT),
            "at": np.ascontiguousarray(
                at.reshape(CPC // 2, H, 2 * F * H)).astype(_NP_DT),
            "bt": np.ascontiguousarray(
                bt.reshape(CPC // 2, W, 2 * F * W)).astype(_NP_DT),
        })
    return in_maps


def _assemble(results):
    out = np.empty((B, C * F, H, W), np.float32)
    for m in range(NCORES):
        # outT[yo, c, h, f, (bl, xo)] -> out[(h,bl), (c,f), xo, yo]
        o = results[m]["outT"].astype(np.float32).reshape(W, CPC, 2, F, 4, H)
        o = o.transpose(2, 4, 1, 3, 5, 0).reshape(B, KPC, H, W)
        out[:, KPC * m:KPC * (m + 1)] = o
    return out


def _run(inputs, trace=False):
    global LAST_RESULT
    nc = _get_nc()
    in_maps = _make_in_maps(**inputs)
    LAST_RESULT = run_bass_kernel_spmd(
        nc, in_maps, list(range(NCORES)), trace=trace
    )
    return _assemble(LAST_RESULT.results)


def kernel(input, x_min, x_max, y_min, y_max):
    return _run({
        "input": np.asarray(input, dtype=np.float32),
        "x_min": np.asarray(x_min, dtype=np.float32),
        "x_max": np.asarray(x_max, dtype=np.float32),
        "y_min": np.asarray(y_min, dtype=np.float32),
        "y_max": np.asarray(y_max, dtype=np.float32),
    })


# revision 22
# speedup vs baseline: 1.1486x; 1.0276x over previous
"""BoxConv2d Trainium2 kernel (8 NeuronCores, SPMD).

Math: the reference's integral-image + fractional box-edge interpolation
pipeline is linear in the input and separable, so per output channel
k = (c, f) it collapses to two dense 128x128 matrix products:

    out[b,k] = A_k @ x[b,c] @ B_k^T

with banded "pixel overlap" matrices
    A_k[xo, a] = clamp(xo - a + x_max_k + 1, 0, 1)
                 - clamp(xo - a + x_min_k, 0, 1)
and likewise B_k for columns.  A/B are built on the host from the tiny
(C,F) box params; the device does pure 128-contraction matmuls.

Sharding: the K = C*F = 128 output channels are split across 8 cores
(16 channels = 4 in_planes per core), so each core reads only its own
4 input planes and input reads are not duplicated chip-wide.

Device dataflow per core (all operands bfloat16, PSUM accum fp32):
  pass 1 (per b,c):   V[j, (f,xo)]  = x_bc^T A^T  (lhsT=x_bc, N=512)
  pass 2 (per c,f,h): O[yo, (bh,xo)] = B_k V      (lhsT=B_k^T, N=512)

Schedule (v2, rebuilt from trace analysis of v1):
  - The PE streams warm back-to-back MMs at ~216ns (N/2.4+2.5); all v1
    losses were elsewhere: input DMA landing at 15.5us starved the PE
    (2.3us stall -> HAM re-throttle -> 630ns cold MMs), output DMA was
    serialized on one queue starting at 21us, and the PSUM->SBUF drains
    (the true steady-state bottleneck: 32768 fp32 columns at ~1ns/col
    split over the only two PSUM-reading engines) ran behind 50 small
    copy instructions.
  - v2: input DMAs are the first instructions, spread over the sync/
    scalar/vector hardware queues so x0+at0 land ~2.5us in; pass-1
    writes b-pairs into 2-bank PSUM tiles and pass-2 (c,f) pairs into
    2-bank tiles (2+2 bufs = all 8 banks, double buffered) so every
    drain moves 1024 columns; drains alternate Vector/Scalar; output
    leaves per (c, f-pair) as 512KB DMAs (4KB lines) on sync (+gpsimd
    for two mid chunks) as soon as both halves drain.  Pass1(c) and
    pass2(c-1) chunks interleave 1:1 to keep the PE warm.
  - Two fp32 dummy matmuls bridge the DMA wait so the HAM full-duty
    grant lands before the real stream.

Numerics: bf16 gives l2 rel error ~3e-3 vs the fp32 reference
(budget 2e-2).
"""

import sys

if "/opt/trn_rl_repo" not in sys.path:
    sys.path.insert(0, "/opt/trn_rl_repo")

import numpy as np
import ml_dtypes

import concourse.bass as bass  # noqa: F401
import concourse.mybir as mybir
import concourse.tile as tile
from concourse import bacc
from concourse.bass_utils import run_bass_kernel_spmd

B, C, F, H, W = 8, 32, 4, 128, 128
NCORES = 8
CPC = C // NCORES   # in_planes per core
KPC = CPC * F       # output channels per core
NP = B // 2         # x batch-pairs per core

_MM_DT = mybir.dt.bfloat16
_NP_DT = ml_dtypes.bfloat16
# box-overlap matrices ride fp8e4m3: entries are exactly 0/1 except two
# fractional edge taps per row, so quantization adds only ~3e-3 global
# error (measured 5.7e-3 total vs 2.9e-3 at bf16, budget 2e-2) while
# halving at/bt DMA bytes; TRN2 matmul accepts mixed bf16/fp8 operands
# (HW-probed: exact vs fp32 reference of the quantized inputs)
_W_DT = mybir.dt.float8e4
_NPW_DT = ml_dtypes.float8_e4m3fn

_NC_CACHE = {}
LAST_RESULT = None


def _build_nc():
    nc = bacc.Bacc(
        "TRN2", target_bir_lowering=False, debug=False, num_devices=NCORES
    )
    # x[p, a, (b2, c, j)]: batch-pairs give 4KB DMA lines
    x_p = nc.declare_dram_parameter(
        "x", [NP, H, 2 * CPC * W], _MM_DT, isOutput=False)
    # at[cp, a, (c2, f, xo)] / bt[cp, j, (c2, f, yo)]: c-pair tiles
    at_p = nc.declare_dram_parameter(
        "at", [CPC // 2, H, 2 * F * H], _W_DT, isOutput=False)
    bt_p = nc.declare_dram_parameter(
        "bt", [CPC // 2, W, 2 * F * W], _W_DT, isOutput=False)
    # transposed output: outT[yo, c, h, f, (bl, xo)] = out[4h+bl, c*F+f, xo, yo]
    # -> per-(c, f-pair, h) DMA writes 2KB contiguous per yo line
    out_p = nc.declare_dram_parameter(
        "outT", [W, CPC, 2, F, B * H // 2], _MM_DT, isOutput=True)

    HB = B * H // 2  # 512: half-batch column count, one PSUM bank

    with tile.TileContext(nc) as tc:
        with (
            tc.tile_pool(name="const", bufs=1) as cpool,
            tc.tile_pool(name="xin", bufs=NP) as xpool,
            tc.tile_pool(name="vall", bufs=4) as vpool,
            tc.tile_pool(name="osb", bufs=4) as opool,
            tc.tile_pool(name="pv", bufs=2, space="PSUM") as pvpool,
            tc.tile_pool(name="po", bufs=2, space="PSUM") as popool,
        ):
            # PSUM->SBUF drains alternate between the two engines that
            # can read PSUM
            eng_i = [0]

            def copy(dst, src):
                if eng_i[0] % 2:
                    nc.scalar.copy(dst, src)
                else:
                    nc.vector.tensor_copy(dst, src)
                eng_i[0] += 1

            # input DMAs are the very first instructions: three hardware
            # queues in parallel, each queue's first transfer is one the
            # pipeline needs first (x0 / x1 / at0).  scalar+vector finish
            # issuing (~1.4us) long before their first drains (~3.5us).
            at_sb = [None] * (CPC // 2)
            bt_sb = [None] * (CPC // 2)
            x_sb = [None] * NP

            def load_x(p, eng):
                x_sb[p] = xpool.tile(
                    [128, 2 * CPC * W], _MM_DT, name=f"xsb{p}", tag="x"
                )
                eng.dma_start(x_sb[p][:], x_p[p])

            def load_at(cp, eng):
                at_sb[cp] = cpool.tile([128, 2 * F * H], _W_DT,
                                       name=f"at{cp}", tag=f"at{cp}")
                eng.dma_start(at_sb[cp][:], at_p[cp])

            def load_bt(cp, eng):
                bt_sb[cp] = cpool.tile([128, 2 * F * W], _W_DT,
                                       name=f"bt{cp}", tag=f"bt{cp}")
                eng.dma_start(bt_sb[cp][:], bt_p[cp])

            # per-ring DMA throughput is only ~110-150GB/s and each ring
            # drains serially: first-needed tiles head the two fast
            # rings (sync/scalar); bt rides the slower gpsimd ring
            # (needed only once pass-2 starts)
            load_at(0, nc.sync)
            load_x(0, nc.scalar)
            load_x(1, nc.sync)
            load_x(2, nc.scalar)
            load_bt(0, nc.gpsimd)
            load_x(3, nc.sync)
            load_at(1, nc.scalar)
            load_bt(1, nc.gpsimd)

            # warm-up: dummy fp32 matmuls (higher switching activity than
            # bf16) start the HAM activity clock during the input DMA
            # window, so the full-duty grant lands as early as possible
            # into the real matmul stream.  memset on Vector (idle until
            # the first drain); dummies target the po pool so the pass-1
            # PSUM path stays clean.
            dum = cpool.tile([128, HB], mybir.dt.float32,
                             name="dum", tag="dum")
            nc.vector.memset(dum[:], 0.0)

            def emit_dummy(i):
                d_ps = popool.tile([128, 2 * HB], mybir.dt.float32,
                                   name=f"dps{i}", tag="po")
                nc.tensor.matmul(
                    d_ps[:, :HB],
                    lhsT=dum[:, :W],
                    rhs=dum[:],
                    start=True,
                    stop=True,
                )

            emit_dummy(0)
            emit_dummy(1)

            v_full = [None] * CPC

            def emit_v_chunk(c, p):
                # pass1: both b's of pair p for plane c into one 2-bank
                # PSUM tile, then a single 1024-col drain scattered into
                # V[j, (f, b, xo)]
                if p == 0:
                    v_full[c] = vpool.tile([128, F * B * H], _MM_DT,
                                           name=f"vall{c}", tag="vall")
                v_ps = pvpool.tile([128, 2 * F * H], mybir.dt.float32,
                                   name=f"vps{c}{p}", tag="pv")
                for hb in range(2):
                    nc.tensor.matmul(
                        v_ps[:, hb * F * H:(hb + 1) * F * H],
                        lhsT=x_sb[p][:, (hb * CPC + c) * W:
                                     (hb * CPC + c + 1) * W],
                        rhs=at_sb[c // 2][:, (c % 2) * F * H:
                                          (c % 2 + 1) * F * H],
                        start=True,
                        stop=True,
                    )
                # dst iterated (b2, f, xo) to match the PSUM layout
                dst = v_full[c][:].rearrange(
                    "p (f b xo) -> p b f xo", f=F, b=B
                )[:, 2 * p:2 * p + 2]
                copy(dst, v_ps[:])

            oq_i = [0]

            def out_eng():
                # odd slots on sync so the tail-critical last chunk rides
                # the fast queue; gpsimd (slower ring) takes even slots
                eng = nc.gpsimd if oq_i[0] % 2 == 0 else nc.sync
                oq_i[0] += 1
                return eng

            def emit_o_chunk(c, fp, h, split_tail=False):
                # pass2: O[yo, (bl,xo)] for batch-half h of the f-pair
                # (fp, fp+1) of plane c; the half-split matters because
                # h=0 depends only on V pairs p0/p1 (i.e. x0/x1), so
                # pass-2 work starts while x2/x3 are still in flight.
                # Two matmuls fill a 2-bank PSUM tile, one 1024-col
                # drain, one 256KB DMA (sync/gpsimd alternate).
                o_ps = popool.tile([128, B * H], mybir.dt.float32,
                                   name=f"ops{c}{fp}{h}", tag="po")
                for f2 in range(2):
                    f = fp + f2
                    nc.tensor.matmul(
                        o_ps[:, f2 * HB:(f2 + 1) * HB],
                        lhsT=bt_sb[c // 2][:, ((c % 2) * F + f) * W:
                                           ((c % 2) * F + f + 1) * W],
                        rhs=v_full[c][:, f * B * H + h * HB:
                                      f * B * H + (h + 1) * HB],
                        start=True,
                        stop=True,
                    )
                seg = opool.tile([128, B * H], _MM_DT,
                                 name=f"osb{c}{fp}{h}", tag="osb")
                if split_tail:
                    # final chunks: drain halves on both engines in
                    # parallel, shortening the post-last-matmul tail
                    nc.vector.tensor_copy(seg[:, :HB], o_ps[:, :HB])
                    nc.scalar.copy(seg[:, HB:], o_ps[:, HB:])
                else:
                    copy(seg[:], o_ps[:])
                out_eng().dma_start(out_p[:, c, h, fp:fp + 2], seg[:])

            # arrival-aware emission: the Tensor stream executes in
            # program order, so chunks are ordered by when their inputs
            # land (x pairs every ~2us, at1 last) and by V half-batch
            # readiness.  O(c,fp,h) needs only V(c) pairs {2h, 2h+1},
            # so pass-2 h=0 chunks start while x2/x3 are in flight, and
            # independent pass-1 work sits between consecutive O chunks
            # to hide the PSUM-recycle latency.
            sched = [
                ("v", 0, 0, 0), ("v", 1, 0, 0), ("d",),
                ("v", 0, 1, 0), ("v", 1, 1, 0), ("d",),
                ("o", 0, 0, 0),
                ("v", 0, 2, 0), ("o", 0, 2, 0),
                ("v", 1, 2, 0), ("o", 1, 0, 0),
                ("v", 0, 3, 0), ("o", 1, 2, 0),
                ("v", 1, 3, 0), ("o", 0, 0, 1),
                ("v", 2, 0, 0), ("o", 0, 2, 1),
                ("v", 2, 1, 0), ("o", 1, 0, 1),
                ("v", 3, 0, 0), ("o", 1, 2, 1),
                ("v", 2, 2, 0), ("o", 2, 0, 0),
                ("v", 3, 1, 0), ("o", 2, 2, 0),
                ("v", 2, 3, 0), ("o", 3, 0, 0),
                ("v", 3, 2, 0), ("o", 2, 0, 1),
                ("v", 3, 3, 0),
            ]
            di = [2]
            for item in sched:
                if item[0] == "v":
                    emit_v_chunk(item[1], item[2])
                elif item[0] == "o":
                    emit_o_chunk(item[1], item[2], item[3])
                else:
                    emit_dummy(di[0])
                    di[0] += 1
            # tail: split drains across both engines so po recycles at
            # half the latency and the final DMAs start sooner
            emit_o_chunk(3, 2, 0, split_tail=True)
            emit_o_chunk(2, 2, 1, split_tail=True)
            emit_o_chunk(3, 0, 1, split_tail=True)
            emit_o_chunk(3, 2, 1, split_tail=True)
    nc.finalize()
    return nc


def _get_nc():
    if "nc" not in _NC_CACHE:
        _NC_CACHE["nc"] = _build_nc()
    return _NC_CACHE["nc"]


def _overlap_mats(lo, hi):
    """(K, out, in) pixel-overlap matrices for a 128-wide axis."""
    t = np.arange(128, dtype=np.float64)
    d = t[:, None] - t[None, :]  # out - in
    lo = lo.astype(np.float64)[:, None, None]
    hi = hi.astype(np.float64)[:, None, None]
    m = np.clip(d[None] + hi + 1.0, 0.0, 1.0) - np.clip(d[None] + lo, 0.0, 1.0)
    return m.astype(np.float32)


def _make_in_maps(input, x_min, x_max, y_min, y_max):
    A = _overlap_mats(x_min.reshape(-1), x_max.reshape(-1))   # (K, xo, a)
    Bm = _overlap_mats(y_min.reshape(-1), y_max.reshape(-1))  # (K, yo, j)
    in_maps = []
    for m in range(NCORES):
        cs = slice(CPC * m, CPC * (m + 1))
        ks = slice(KPC * m, KPC * (m + 1))
        # x[p, a, (b2, c, j)]
        xm = input[:, cs]                                   # [b, c, a, j]
        xm = xm.reshape(NP, 2, CPC, H, W)
        xm = xm.transpose(0, 3, 1, 2, 4).reshape(NP, H, 2 * CPC * W)
        # at[cp, a, (c2, f, xo)] = A[k=(cp*2+c2)*F+f, xo, a]
        at = A[ks].reshape(CPC // 2, 2, F, H, H).transpose(0, 4, 1, 2, 3)
        bt = Bm[ks].reshape(CPC // 2, 2, F, W, W).transpose(0, 4, 1, 2, 3)
        in_maps.append({
            "x": np.ascontiguousarray(xm).astype(_NP_DT),
            "at": np.ascontiguousarray(
                at.reshape(CPC // 2, H, 2 * F * H)).astype(_NPW_DT),
            "bt": np.ascontiguousarray(
                bt.reshape(CPC // 2, W, 2 * F * W)).astype(_NPW_DT),
        })
    return in_maps


def _assemble(results):
    out = np.empty((B, C * F, H, W), np.float32)
    for m in range(NCORES):
        # outT[yo, c, h, f, (bl, xo)] -> out[(h,bl), (c,f), xo, yo]
        o = results[m]["outT"].astype(np.float32).reshape(W, CPC, 2, F, 4, H)
        o = o.transpose(2, 4, 1, 3, 5, 0).reshape(B, KPC, H, W)
        out[:, KPC * m:KPC * (m + 1)] = o
    return out


def _run(inputs, trace=False):
    global LAST_RESULT
    nc = _get_nc()
    in_maps = _make_in_maps(**inputs)
    LAST_RESULT = run_bass_kernel_spmd(
        nc, in_maps, list(range(NCORES)), trace=trace
    )
    return _assemble(LAST_RESULT.results)


def kernel(input, x_min, x_max, y_min, y_max):
    return _run({
        "input": np.asarray(input, dtype=np.float32),
        "x_min": np.asarray(x_min, dtype=np.float32),
        "x_max": np.asarray(x_max, dtype=np.float32),
        "y_min": np.asarray(y_min, dtype=np.float32),
        "y_max": np.asarray(y_max, dtype=np.float32),
    })
